# revision 19
# baseline (speedup 1.0000x reference)
"""Causal multi-head attention (B=8, S=1024, D=768, H=12, Dh=64) on 8 TRN2
NeuronCores, batch-parallel (one batch element per core).

Per-core Bass/Tile kernel, structured for engine overlap:
  - x DMAs ride the SP HWDGE ring while W DMAs ride the ACT ring in parallel.
  - Per s-chunk: PE transposes x -> x^T (bf16) in batches of 3 per PSUM bank
    (one DVE copy per batch), then immediately projects V chunks so PE starts
    ~2us into the kernel.
  - Per head-pair group g: Q^T/K^T projections (weight-pair stationary, x^T
    moving) accumulate into a single 2-bank [P,1024] PSUM tile (one DVE copy),
    then attention for the two heads — ScalarE exp work of group g overlaps
    the PE projection work of group g+1.
  - Scores are computed transposed S^T[t, s] = K·Q^T with causal skip into a
    2-bank [P,1024] PSUM tile; ONE exp per key-chunk on ScalarE (scale=1/8
    folded in, no max subtraction — scores are O(5)); diagonal block masked by
    a 0/1 triangle multiply on GpSimd (keeps DVE free).
  - V' carries a ones-column per head so the PV matmul accumulates softmax
    denominators in ctx^T row 64; per 4 s-chunks: 4 PE transposes into one
    PSUM bank + one strided reciprocal + one broadcast multiply normalize
    straight into the output layout.
"""

import sys
from contextlib import ExitStack

for _p in ("/opt/trn_rl_repo", "/root/.axon_site/_ro/trn_rl_repo"):
    if _p not in sys.path:
        sys.path.append(_p)

import numpy as np

import concourse.bass as bass  # noqa: F401
import concourse.bacc as bacc
import concourse.mybir as mybir
import concourse.tile as tile
from concourse.bass import ts
from concourse.bass_utils import run_bass_kernel_spmd
from concourse.masks import make_identity, make_upper_triangular

FP32 = mybir.dt.float32
BF16 = mybir.dt.bfloat16

B, S, D, H, DH = 8, 1024, 768, 12, 64
P = 128
NS, NK = S // P, D // P  # 8 s-chunks, 6 k-tiles
NG = H // 2              # 6 head-pair groups
VW = DH + 1              # 65: V columns + ones column
N_CORES = 8


def _build_tile_kernel(tc, outs, ins):
    nc = tc.nc
    x, Wq, Wk, Wv = ins["x"], ins["Wq"], ins["Wk"], ins["Wv"]
    out = outs["out"]

    x_t = x.rearrange("(ns p) d -> p ns d", p=P)
    out_t = out.rearrange("(ns p) d -> p ns d", p=P)

    ctx = ExitStack()
    with ctx:
        consts = ctx.enter_context(tc.tile_pool(name="consts", bufs=1))
        sb1 = ctx.enter_context(tc.tile_pool(name="sb1", bufs=1))
        win = ctx.enter_context(tc.tile_pool(name="win", bufs=4))
        xin = ctx.enter_context(tc.tile_pool(name="xin", bufs=8))
        ptp = ctx.enter_context(tc.tile_pool(name="ptp", bufs=6))
        ctxs = ctx.enter_context(tc.tile_pool(name="ctxs", bufs=2))
        recp = ctx.enter_context(tc.tile_pool(name="recp", bufs=4))
        # PSUM: sc 2x2 + acc 1x2 + ctx 2x1 = 8 banks exactly. The sc slots
        # host score pairs, x-transpose batches, V projection accumulators and
        # norm transposes; acc is the QK projection accumulator (own slot so
        # the score rotation never waits on a projection copy); ctx slots are
        # per-head per-s-half PV accumulators (first half frees mid-pair).
        ps_sc = ctx.enter_context(tc.tile_pool(name="ps_sc", bufs=2, space="PSUM"))
        ps_acc = ctx.enter_context(tc.tile_pool(name="ps_acc", bufs=1, space="PSUM"))
        ps_ctx = ctx.enter_context(tc.tile_pool(name="ps_ctx", bufs=2, space="PSUM"))

        ident = consts.tile([P, P], FP32)
        make_identity(nc, ident)
        identb = consts.tile([VW, VW], BF16)
        make_identity(nc, identb)
        maskT = consts.tile([P, P], BF16)
        make_upper_triangular(nc, maskT, val=1.0, diag=True)

        xT = sb1.tile([P, NK, S], BF16)
        Wq_sb = sb1.tile([P, NK // 2, 2, H, DH], BF16)
        Wk_sb = sb1.tile([P, NK // 2, 2, H, DH], BF16)
        Wv_sb = sb1.tile([P, NK // 2, 2, H, DH], BF16)
        QT = sb1.tile([P, NG, S], BF16)
        KT = sb1.tile([P, NG, S], BF16)
        Vp = sb1.tile([P, NS, H * VW], BF16)
        out_sb = sb1.tile([P, NS, D], FP32)

        nc.gpsimd.memset(
            Vp.rearrange("p ns (h w) -> p ns h w", w=VW)[:, :, :, DH:VW], 1.0
        )

        def load_w_chunk(w_dram, w_sb, kt2, h0, h1):
            # Two consecutive D-rows per partition line: 512B-contiguous on
            # both DMA sides (full SDMA rate; <512B runs pay a 2x penalty).
            # Contraction K-tile (kt2, two) maps partition p to D-row
            # kt2*256 + 2p + two; x^T uses the same permuted order.
            nh = h1 - h0
            wtmp = win.tile([P, H // 2, 2 * DH], FP32, tag="w")
            # W DMAs ride the ACT HWDGE ring (x rides the SP ring)
            nc.scalar.dma_start(
                out=wtmp[:, 0:nh, :],
                in_=w_dram[h0:h1, kt2 * 256 : (kt2 + 1) * 256, :].rearrange(
                    "h (p two) d -> p h (two d)", two=2
                ),
            )
            # f32 -> bf16 cast (Wv on DVE — fast, needed first for V proj;
            # Wq/Wk alternate Pool / DVE); also reshuffles to [kt2, two, h, d]
            # so matmul slices for a K-tile (kt2, two) are contiguous.
            if w_dram is Wv:
                eng = nc.vector
            else:
                eng = nc.gpsimd if (kt2 % 2 == 0) else nc.vector
            eng.tensor_copy(
                out=w_sb[:, kt2, :, h0:h1, :],
                in_=wtmp[:, 0:nh, :].rearrange("p h (two d) -> p two h d", two=2),
            )

        # Moderately sized W DMAs (per-DMA HWDGE overhead is ~0.6us), in
        # availability order: all three Wv K-tiles first (V proj runs first),
        # then Wq/Wk, first-half heads before second-half. x chunks ride the
        # SP ring, W the ACT ring, so descriptor generation overlaps.
        xcs = []
        for ns in range(NS):
            xc = xin.tile([P, D], FP32, tag="xc")
            nc.sync.dma_start(out=xc, in_=x_t[:, ns, :])
            xcs.append(xc)
        w_order = [(Wv, Wv_sb, kt2) for kt2 in range(3)] + [
            (w, w_sb, kt2)
            for kt2 in range(3)
            for w, w_sb in ((Wq, Wq_sb), (Wk, Wk_sb))
        ]
        for w_dram, w_sb, kt2 in w_order:
            load_w_chunk(w_dram, w_sb, kt2, 0, 6)
        for w_dram, w_sb, kt2 in w_order:
            load_w_chunk(w_dram, w_sb, kt2, 6, 12)

        # x transposes (permuted-D order to match the W layout), batched 6
        # per 2-bank PSUM slot (3 per bank) so each ns needs only one DVE copy.
        for ns in range(NS):
            xcv = xcs[ns].rearrange("p (kt2 q two) -> p kt2 two q", kt2=3, two=2)
            xtp = ps_sc.tile([P, 1024], FP32, tag="sc", name="xtp")
            for kt in range(NK):
                kt2, two = divmod(kt, 2)
                col = (kt // 3) * 512 + (kt % 3) * P
                nc.tensor.transpose(
                    xtp[:, col : col + P], xcv[:, kt2, two, :], ident
                )
            nc.vector.tensor_copy(
                xT[:, 0:NK, ts(ns, P)].rearrange("p (b k) q -> p b k q", b=2),
                xtp.rearrange("p (b r) -> p b r", b=2)[:, :, 0 : 3 * P].rearrange(
                    "p b (k q) -> p b k q", k=3
                ),
            )

        # ---- emission units for the software-pipelined main loop ----

        def vproj_unit(hf, ns):
            # half hf covers heads 6*hf .. 6*hf+5 (384 columns, one PSUM bank);
            # stationary x^T block reused across both halves' matmuls by the
            # caller pairing (same kt order).
            def emit():
                accv = ps_sc.tile([P, 1024], FP32, tag="sc", name="accv")
                for kt in range(NK):
                    kt2, two = divmod(kt, 2)
                    nc.tensor.matmul(
                        accv[:, 0:384],
                        xT[:, kt, ts(ns, P)],
                        Wv_sb[:, kt2, two, 6 * hf : 6 * hf + 6, :],
                        start=(kt == 0),
                        stop=(kt == NK - 1),
                    )
                nc.vector.tensor_copy(
                    Vp.rearrange("p ns (h w) -> p ns h w", w=VW)[
                        :, ns, 6 * hf : 6 * hf + 6, 0:DH
                    ],
                    accv[:, 0:384].rearrange("p (h d) -> p h d", d=DH),
                )

            return emit

        def qkproj_unit(g, w_sb, dstT):
            def emit():
                acc = ps_acc.tile([P, 1024], FP32, tag="acc", name="acc")
                for kt in range(NK):
                    kt2, two = divmod(kt, 2)
                    for c in range(2):
                        nc.tensor.matmul(
                            acc[:, ts(c, 512)],
                            w_sb[:, kt2, two, 2 * g : 2 * g + 2, :],
                            xT[:, kt, ts(c, 512)],
                            start=(kt == 0),
                            stop=(kt == NK - 1),
                        )
                for c in range(2):
                    nc.vector.tensor_copy(dstT[:, g, ts(c, 512)], acc[:, ts(c, 512)])

            return emit

        def proj_units(g):
            units = []
            if g == 0:
                units += [vproj_unit(0, ns) for ns in range(NS)]
            elif g == 3:
                units += [vproj_unit(1, ns) for ns in range(NS)]
            for w_sb, dstT in ((Wq_sb, QT), (Wk_sb, KT)):
                units.append(qkproj_unit(g, w_sb, dstT))
            return units

        def attention_pair_units(g):
            """Returns (stream_units, norm_units) for head pair (2g, 2g+1).

            Scores for both heads of the pair are computed by adjacent K=64
            matmuls at lhsT base partitions 0/64 — auto-derived PE row tiling
            runs them concurrently in disjoint row groups on hardware. The
            s-axis is processed in two passes (columns [s0,512) then
            [max(512,s0),1024)) so each head's PV accumulator is a single
            PSUM bank per pass; the first half's ctx is copied out mid-pair.
            Score/exp units run one step ahead of PV units so PE's in-order
            queue always has independent matmuls while exp + mask drain.
            """
            hA, hB = 2 * g, 2 * g + 1
            state = {}

            def seg(pss, j):
                s0 = j * P
                lo = s0 if pss == 1 else max(512, s0)
                hi = 512 if pss == 1 else S
                return s0, lo, hi

            def smm_unit(pss, j):
                def emit():
                    s0, lo, hi = seg(pss, j)
                    cw = hi - lo
                    ptile = ptp.tile([P, S], BF16, tag="pt", name="ptile")
                    state[(pss, j)] = ptile
                    sc = ps_sc.tile([P, 1024], FP32, tag="sc", name="scs")
                    for po, off in ((0, 0), (DH, 512)):
                        nc.tensor.matmul(
                            sc[:, off : off + cw],
                            KT[po : po + DH, g, ts(j, P)],
                            QT[po : po + DH, g, lo:hi],
                            start=True,
                            stop=True,
                        )
                    # one exp for both heads' chunks (segmented AP skips the
                    # pad columns between A's chunk and B's bank-aligned chunk)
                    nc.scalar.activation(
                        out=ptile.rearrange("p (b c) -> p b c", b=2)[:, :, 0:cw],
                        in_=sc.rearrange("p (b c) -> p b c", b=2)[:, :, 0:cw],
                        func=mybir.ActivationFunctionType.Exp,
                        scale=0.125,
                    )
                    if lo == s0:  # this chunk starts at the causal diagonal
                        nc.gpsimd.tensor_mul(ptile[:, 0:P], ptile[:, 0:P], maskT)
                        nc.gpsimd.tensor_mul(
                            ptile[:, 512 : 512 + P], ptile[:, 512 : 512 + P], maskT
                        )

                return emit

            def pv_unit(pss, h, j):
                hb = h % 2

                def emit():
                    s0, lo, hi = seg(pss, j)
                    cw = hi - lo
                    key = ("ctx", pss, hb)
                    if j == 0:
                        state[key] = ps_ctx.tile(
                            [VW, 512], FP32, tag="ctx", name="ctxps"
                        )
                    co = 0 if pss == 1 else 512
                    nc.tensor.matmul(
                        state[key][:, lo - co : hi - co],
                        Vp[:, j, h * VW : (h + 1) * VW],
                        state[(pss, j)][:, 512 * hb : 512 * hb + cw],
                        start=(j == 0),
                        stop=(j == (3 if pss == 1 else NS - 1)),
                        skip_group_check=True,
                    )

                return emit

            def ctx_copy_unit(pss, h):
                hb = h % 2

                def emit():
                    if pss == 1:
                        state[("sb", hb)] = ctxs.tile(
                            [VW, S], BF16, tag="ctxs", name="ctxsb"
                        )
                    nc.vector.tensor_copy(
                        state[("sb", hb)][:, ts(pss - 1, 512)],
                        state[("ctx", pss, hb)],
                    )

                return emit

            def norm_unit(h, m0):
                hb = h % 2

                def emit():
                    # 4 transposed s-chunks into one PSUM bank, then one
                    # strided reciprocal + one broadcast multiply.
                    trm = ps_sc.tile([P, 1024], BF16, tag="sc", name="trm")
                    for i in range(4):
                        nc.tensor.transpose(
                            trm[:, i * VW : (i + 1) * VW],
                            state[("sb", hb)][:, ts(m0 + i, P)],
                            identb,
                        )
                    trv = trm[:, 0 : 4 * VW].rearrange("p (m w) -> p m w", w=VW)
                    rec = recp.tile([P, 4, 1], FP32, tag="rec")

                    nc.vector.reciprocal(rec, trv[:, :, DH:VW])
                    nc.vector.tensor_mul(
                        out_sb[:, m0 : m0 + 4, h * DH : (h + 1) * DH],
                        trv[:, :, 0:DH],
                        rec.broadcast_to([P, 4, DH]),
                    )

                return emit

            order = [(1, j) for j in range(4)] + [(2, j) for j in range(NS)]
            stream = [smm_unit(*order[0]), smm_unit(*order[1])]
            for i in range(2, len(order)):
                pp, jj = order[i - 2]
                stream += [pv_unit(pp, hA, jj), pv_unit(pp, hB, jj)]
                if (pp, jj) == (1, 3):
                    stream += [ctx_copy_unit(1, hA), ctx_copy_unit(1, hB)]
                stream.append(smm_unit(*order[i]))
            for pp, jj in order[-2:]:
                stream += [pv_unit(pp, hA, jj), pv_unit(pp, hB, jj)]
            stream += [ctx_copy_unit(2, hA), ctx_copy_unit(2, hB)]
            norms = [
                norm_unit(hA, 0),
                norm_unit(hA, 4),
                norm_unit(hB, 0),
                norm_unit(hB, 4),
            ]
            return stream, norms

        # Software pipeline: group g's projections emit interleaved with
        # group g-1's attention so ScalarE exp always overlaps PE matmuls;
        # each pair's normalization is deferred into the next pair's stream.
        pending_norms = []
        for gi in range(NG + 1):
            att = []
            if gi >= 1:
                stream, norms = attention_pair_units(gi - 1)
                # fold the previous pair's norm units into this stream
                for k, nu in enumerate(pending_norms):
                    stream.insert(4 + 5 * k, nu)
                att = stream
                pending_norms = norms
            prj = proj_units(gi) if gi < NG else []
            # proportional round-robin merge
            na, np_ = len(att), len(prj)
            ia = ip = 0
            while ia < na or ip < np_:
                if ip * max(na, 1) <= ia * max(np_, 1):
                    if ip < np_:
                        prj[ip]()
                        ip += 1
                    else:
                        att[ia]()
                        ia += 1
                else:
                    if ia < na:
                        att[ia]()
                        ia += 1
                    else:
                        prj[ip]()
                        ip += 1
        for nu in pending_norms:
            nu()

        # output DMAs spread over three HWDGE rings (all idle by now) so the
        # final flush isn't serialized on one ring's descriptor generation
        rings = (nc.sync, nc.scalar, nc.gpsimd)
        di = 0
        for c0 in (0, 6 * DH):
            for ns in range(NS):
                rings[di % 3].dma_start(
                    out=out_t[:, ns, c0 : c0 + 6 * DH],
                    in_=out_sb[:, ns, c0 : c0 + 6 * DH],
                )
                di += 1


_NC = {}


def build_nc(reps=1):
    """Build + compile the per-core Bass program once per process.

    reps > 1 emits the body multiple times with all-engine barriers between
    repetitions — used only for marginal-time measurement in test harnesses.
    """
    if reps in _NC:
        return _NC[reps]
    nc = bacc.Bacc("TRN2", target_bir_lowering=False, debug=False)
    ins = {
        "x": nc.dram_tensor("x", [S, D], FP32, kind="ExternalInput").ap(),
        "Wq": nc.dram_tensor("Wq", [H, D, DH], FP32, kind="ExternalInput").ap(),
        "Wk": nc.dram_tensor("Wk", [H, D, DH], FP32, kind="ExternalInput").ap(),
        "Wv": nc.dram_tensor("Wv", [H, D, DH], FP32, kind="ExternalInput").ap(),
    }
    outs = {"out": nc.dram_tensor("out", [S, D], FP32, kind="ExternalOutput").ap()}
    with tile.TileContext(nc) as tc:
        for i in range(reps):
            if i:
                tc.strict_bb_all_engine_barrier()
            _build_tile_kernel(tc, outs, ins)
    nc.compile()
    _NC[reps] = nc
    return nc


def make_in_maps(x, Wq, Wk, Wv):
    x = np.ascontiguousarray(x, dtype=np.float32)
    Wq = np.ascontiguousarray(Wq, dtype=np.float32)
    Wk = np.ascontiguousarray(Wk, dtype=np.float32)
    Wv = np.ascontiguousarray(Wv, dtype=np.float32)
    return [
        {"x": np.ascontiguousarray(x[b]), "Wq": Wq, "Wk": Wk, "Wv": Wv}
        for b in range(B)
    ]


def kernel(x, Wq, Wk, Wv):
    nc = build_nc()
    res = run_bass_kernel_spmd(nc, make_in_maps(x, Wq, Wk, Wv), list(range(N_CORES)))
    return np.stack([res.results[b]["out"] for b in range(B)], axis=0)


# revision 39
# speedup vs baseline: 1.4244x; 1.4244x over previous
"""Causal multi-head attention (B=8, S=1024, D=768, H=12, Dh=64) on 8 TRN2
NeuronCores, batch-parallel (one batch element per core).

Per-core Bass/Tile kernel, structured for engine overlap:
  - Input DMAs in availability order: the first x chunks ride the ACT ring
    ahead of W (so x lands early), the rest stream on the SP ring in parallel
    with W; all Wv K-tiles precede Wq/Wk (V projection runs first).
  - Per s-chunk: PE transposes x -> x^T (bf16), 6 transposes batched per
    2-bank PSUM slot so each chunk needs one DVE copy; V projection follows
    as chunks land so PE starts ~2us into the kernel.
  - Per head-pair group g: Q^T/K^T projections (weight-pair stationary, x^T
    moving) accumulate into a dedicated 2-bank PSUM slot, interleaved with
    group g-1's attention so ScalarE exp always overlaps PE matmuls.
  - Scores are computed transposed S^T[t, s] = K·Q^T with causal skip, both
    heads of the pair per unit: adjacent K=64 matmuls at lhsT base partitions
    0/64 (PE row tiling) write the two banks of one PSUM slot; ONE ScalarE
    exp with a segmented AP covers both heads (scale=1/8 folded in, no max
    subtraction — scores are O(5)); diagonal blocks masked by 0/1 triangle
    multiplies on GpSimd (keeps DVE free).
  - Score/exp units run one emission step ahead of PV units so PE's in-order
    queue always has independent matmuls while exp + mask drain; the s-axis
    is processed in two passes so each head's PV accumulator is one PSUM bank
    per pass (ones-column in V' accumulates softmax denominators in row 64);
    the first half's ctx is copied out (bf16) mid-pair.
  - Normalization per 4 s-chunks: 4 PE transposes (bf16, FWL) into one PSUM
    bank + one strided reciprocal + one broadcast multiply straight into the
    output layout; each pair's normalization is deferred into the next pair's
    stream; Q/K projection matmuls are front-loaded at each pair's start to
    fill the PE bubble while the first exp chains drain; output DMAs are
    split (heads 0-5 / 6-9 / 10-11) over three DGE rings so the flush gated
    on the final pair is only 128 columns.
"""

import sys
from contextlib import ExitStack

for _p in ("/opt/trn_rl_repo", "/root/.axon_site/_ro/trn_rl_repo"):
    if _p not in sys.path:
        sys.path.append(_p)

import numpy as np

import concourse.bass as bass  # noqa: F401
import concourse.bacc as bacc
import concourse.mybir as mybir
import concourse.tile as tile
from concourse.bass import ts
from concourse.bass_utils import run_bass_kernel_spmd
from concourse.masks import make_identity, make_upper_triangular

FP32 = mybir.dt.float32
BF16 = mybir.dt.bfloat16

B, S, D, H, DH = 8, 1024, 768, 12, 64
P = 128
NS, NK = S // P, D // P  # 8 s-chunks, 6 k-tiles
NG = H // 2              # 6 head-pair groups
VW = DH + 1              # 65: V columns + ones column
N_CORES = 8


def _build_tile_kernel(tc, outs, ins):
    nc = tc.nc
    x, Wq, Wk, Wv = ins["x"], ins["Wq"], ins["Wk"], ins["Wv"]
    out = outs["out"]

    x_t = x.rearrange("(ns p) d -> p ns d", p=P)
    out_t = out.rearrange("(ns p) d -> p ns d", p=P)

    ctx = ExitStack()
    with ctx:
        consts = ctx.enter_context(tc.tile_pool(name="consts", bufs=1))
        sb1 = ctx.enter_context(tc.tile_pool(name="sb1", bufs=1))
        win = ctx.enter_context(tc.tile_pool(name="win", bufs=4))
        xin = ctx.enter_context(tc.tile_pool(name="xin", bufs=8))
        ptp = ctx.enter_context(tc.tile_pool(name="ptp", bufs=6))
        ctxs = ctx.enter_context(tc.tile_pool(name="ctxs", bufs=2))
        recp = ctx.enter_context(tc.tile_pool(name="recp", bufs=6))
        # PSUM: sc 2x2 + acc 1x2 + ctx 2x1 = 8 banks exactly. The sc slots
        # host score pairs, x-transpose batches, V projection accumulators and
        # norm transposes; acc is the QK projection accumulator (own slot so
        # the score rotation never waits on a projection copy); ctx slots are
        # per-head per-s-half PV accumulators (first half frees mid-pair).
        ps_sc = ctx.enter_context(tc.tile_pool(name="ps_sc", bufs=2, space="PSUM"))
        ps_acc = ctx.enter_context(tc.tile_pool(name="ps_acc", bufs=1, space="PSUM"))
        ps_ctx = ctx.enter_context(tc.tile_pool(name="ps_ctx", bufs=2, space="PSUM"))

        ident = consts.tile([P, P], FP32)
        make_identity(nc, ident)
        identb = consts.tile([VW, VW], BF16)
        make_identity(nc, identb)
        maskT = consts.tile([P, P], BF16)
        make_upper_triangular(nc, maskT, val=1.0, diag=True)

        xT = sb1.tile([P, NK, S], BF16)
        Wq_sb = sb1.tile([P, NK // 2, 2, H, DH], BF16)
        Wk_sb = sb1.tile([P, NK // 2, 2, H, DH], BF16)
        Wv_sb = sb1.tile([P, NK // 2, 2, H, DH], BF16)
        QT = sb1.tile([P, NG, S], BF16)
        KT = sb1.tile([P, NG, S], BF16)
        Vp = sb1.tile([P, NS, H * VW], BF16)
        out_sb = sb1.tile([P, NS, D], FP32)

        nc.gpsimd.memset(
            Vp.rearrange("p ns (h w) -> p ns h w", w=VW)[:, :, :, DH:VW], 1.0
        )

        def load_w_chunk(w_dram, w_sb, kt2, h0, h1):
            # Two consecutive D-rows per partition line: 512B-contiguous on
            # both DMA sides (full SDMA rate; <512B runs pay a 2x penalty).
            # Contraction K-tile (kt2, two) maps partition p to D-row
            # kt2*256 + 2p + two; x^T uses the same permuted order.
            nh = h1 - h0
            wtmp = win.tile([P, H // 2, 2 * DH], FP32, tag="w")
            # W DMAs ride the ACT HWDGE ring (x rides the SP ring)
            nc.scalar.dma_start(
                out=wtmp[:, 0:nh, :],
                in_=w_dram[h0:h1, kt2 * 256 : (kt2 + 1) * 256, :].rearrange(
                    "h (p two) d -> p h (two d)", two=2
                ),
            )
            # f32 -> bf16 cast (Wv on DVE — fast, needed first for V proj;
            # Wq/Wk alternate Pool / DVE); also reshuffles to [kt2, two, h, d]
            # so matmul slices for a K-tile (kt2, two) are contiguous.
            if w_dram is Wv:
                eng = nc.vector
            else:
                eng = nc.gpsimd if (kt2 % 2 == 0) else nc.vector
            eng.tensor_copy(
                out=w_sb[:, kt2, :, h0:h1, :],
                in_=wtmp[:, 0:nh, :].rearrange("p h (two d) -> p two h d", two=2),
            )

        # Moderately sized W DMAs (per-DMA HWDGE overhead is ~0.6us), in
        # availability order: all three Wv K-tiles first (V proj runs first),
        # then Wq/Wk, first-half heads before second-half. x chunks ride the
        # SP ring, W the ACT ring, so descriptor generation overlaps.
        xcs = []
        for ns in range(NS):
            xc = xin.tile([P, D], FP32, tag="xc")
            # first chunks ride the ACT ring ahead of W so x lands early;
            # the rest stream on the SP ring in parallel with W
            eng = nc.scalar if ns < 3 else nc.sync
            eng.dma_start(out=xc, in_=x_t[:, ns, :])
            xcs.append(xc)
        w_order = [(Wv, Wv_sb, kt2) for kt2 in range(3)] + [
            (w, w_sb, kt2)
            for kt2 in range(3)
            for w, w_sb in ((Wq, Wq_sb), (Wk, Wk_sb))
        ]
        for w_dram, w_sb, kt2 in w_order:
            load_w_chunk(w_dram, w_sb, kt2, 0, 6)
        for w_dram, w_sb, kt2 in w_order:
            load_w_chunk(w_dram, w_sb, kt2, 6, 12)

        # x transposes (permuted-D order to match the W layout), batched 6
        # per 2-bank PSUM slot (3 per bank) so each ns needs only one DVE copy.
        for ns in range(NS):
            xcv = xcs[ns].rearrange("p (kt2 q two) -> p kt2 two q", kt2=3, two=2)
            xtp = ps_sc.tile([P, 1024], FP32, tag="sc", name="xtp")
            for kt in range(NK):
                kt2, two = divmod(kt, 2)
                col = (kt // 3) * 512 + (kt % 3) * P
                nc.tensor.transpose(
                    xtp[:, col : col + P], xcv[:, kt2, two, :], ident
                )
            nc.vector.tensor_copy(
                xT[:, 0:NK, ts(ns, P)].rearrange("p (b k) q -> p b k q", b=2),
                xtp.rearrange("p (b r) -> p b r", b=2)[:, :, 0 : 3 * P].rearrange(
                    "p b (k q) -> p b k q", k=3
                ),
            )

        # ---- emission units for the software-pipelined main loop ----

        def vproj_unit(hf, ns):
            # half hf covers heads 6*hf .. 6*hf+5 (384 columns, one PSUM bank);
            # stationary x^T block reused across both halves' matmuls by the
            # caller pairing (same kt order).
            def emit():
                accv = ps_sc.tile([P, 1024], FP32, tag="sc", name="accv")
                for kt in range(NK):
                    kt2, two = divmod(kt, 2)
                    nc.tensor.matmul(
                        accv[:, 0:384],
                        xT[:, kt, ts(ns, P)],
                        Wv_sb[:, kt2, two, 6 * hf : 6 * hf + 6, :],
                        start=(kt == 0),
                        stop=(kt == NK - 1),
                    )
                nc.vector.tensor_copy(
                    Vp.rearrange("p ns (h w) -> p ns h w", w=VW)[
                        :, ns, 6 * hf : 6 * hf + 6, 0:DH
                    ],
                    accv[:, 0:384].rearrange("p (h d) -> p h d", d=DH),
                )

            return emit

        def qkproj_unit(g, w_sb, dstT):
            def emit():
                # g=0 bootstrap: K-proj on an sc slot so it doesn't wait for
                # Q-proj's copies through the single acc slot (attention
                # hasn't started yet, sc slots are idle)
                if g == 0 and w_sb is Wk_sb:
                    acc = ps_sc.tile([P, 1024], FP32, tag="sc", name="acc")
                else:
                    acc = ps_acc.tile([P, 1024], FP32, tag="acc", name="acc")
                for kt in range(NK):
                    kt2, two = divmod(kt, 2)
                    for c in range(2):
                        nc.tensor.matmul(
                            acc[:, ts(c, 512)],
                            w_sb[:, kt2, two, 2 * g : 2 * g + 2, :],
                            xT[:, kt, ts(c, 512)],
                            start=(kt == 0),
                            stop=(kt == NK - 1),
                        )
                for c in range(2):
                    nc.vector.tensor_copy(dstT[:, g, ts(c, 512)], acc[:, ts(c, 512)])

            return emit

        def proj_units(g):
            units = []
            if g == 0:
                units += [vproj_unit(0, ns) for ns in range(NS)]
            elif g == 3:
                units += [vproj_unit(1, ns) for ns in range(NS)]
            for w_sb, dstT in ((Wq_sb, QT), (Wk_sb, KT)):
                units.append(qkproj_unit(g, w_sb, dstT))
            return units

        def attention_pair_units(g):
            """Returns (stream_units, norm_units) for head pair (2g, 2g+1).

            Scores for both heads of the pair are computed by adjacent K=64
            matmuls at lhsT base partitions 0/64 — auto-derived PE row tiling
            runs them concurrently in disjoint row groups on hardware. The
            s-axis is processed in two passes (columns [s0,512) then
            [max(512,s0),1024)) so each head's PV accumulator is a single
            PSUM bank per pass; the first half's ctx is copied out mid-pair.
            Score/exp units run one step ahead of PV units so PE's in-order
            queue always has independent matmuls while exp + mask drain.
            """
            hA, hB = 2 * g, 2 * g + 1
            state = {}

            def seg(pss, j):
                s0 = j * P
                lo = s0 if pss == 1 else max(512, s0)
                hi = 512 if pss == 1 else S
                return s0, lo, hi

            def smm_unit(pss, j):
                def emit():
                    s0, lo, hi = seg(pss, j)
                    cw = hi - lo
                    ptile = ptp.tile([P, S], BF16, tag="pt", name="ptile")
                    state[(pss, j)] = ptile
                    sc = ps_sc.tile([P, 1024], FP32, tag="sc", name="scs")
                    for po, off in ((0, 0), (DH, 512)):
                        nc.tensor.matmul(
                            sc[:, off : off + cw],
                            KT[po : po + DH, g, ts(j, P)],
                            QT[po : po + DH, g, lo:hi],
                            start=True,
                            stop=True,
                        )
                    # one exp for both heads' chunks (segmented AP skips the
                    # pad columns between A's chunk and B's bank-aligned chunk)
                    nc.scalar.activation(
                        out=ptile.rearrange("p (b c) -> p b c", b=2)[:, :, 0:cw],
                        in_=sc.rearrange("p (b c) -> p b c", b=2)[:, :, 0:cw],
                        func=mybir.ActivationFunctionType.Exp,
                        scale=0.125,
                    )
                    if lo == s0:  # this chunk starts at the causal diagonal
                        nc.gpsimd.tensor_mul(ptile[:, 0:P], ptile[:, 0:P], maskT)
                        nc.gpsimd.tensor_mul(
                            ptile[:, 512 : 512 + P], ptile[:, 512 : 512 + P], maskT
                        )

                return emit

            def pv_unit(pss, h, j):
                hb = h % 2

                def emit():
                    s0, lo, hi = seg(pss, j)
                    cw = hi - lo
                    key = ("ctx", pss, hb)
                    if j == 0:
                        state[key] = ps_ctx.tile(
                            [VW, 512], FP32, tag="ctx", name="ctxps"
                        )
                    co = 0 if pss == 1 else 512
                    nc.tensor.matmul(
                        state[key][:, lo - co : hi - co],
                        Vp[:, j, h * VW : (h + 1) * VW],
                        state[(pss, j)][:, 512 * hb : 512 * hb + cw],
                        start=(j == 0),
                        stop=(j == (3 if pss == 1 else NS - 1)),
                        skip_group_check=True,
                    )

                return emit

            def ctx_copy_unit(pss, h):
                hb = h % 2

                def emit():
                    if pss == 1:
                        state[("sb", hb)] = ctxs.tile(
                            [VW, S], BF16, tag="ctxs", name="ctxsb"
                        )
                    # final pair, final pass: head B's copy rides ScalarE
                    # (no exps remain) so both heads' ctx drains in parallel
                    # and the tail norm chain starts ~0.6us sooner
                    if g == NG - 1 and pss == 2 and hb == 1:
                        eng = nc.scalar.copy
                    else:
                        eng = nc.vector.tensor_copy
                    eng(
                        state[("sb", hb)][:, ts(pss - 1, 512)],
                        state[("ctx", pss, hb)],
                    )

                return emit

            def norm_unit(h, m0):
                hb = h % 2

                def emit():
                    # 4 transposed s-chunks into one PSUM bank, then one
                    # strided reciprocal + one broadcast multiply.
                    trm = ps_ctx.tile([P, 1024], BF16, tag="ctx", name="trm")
                    for i in range(4):
                        nc.tensor.transpose(
                            trm[:, i * (VW + 1) : i * (VW + 1) + VW],
                            state[("sb", hb)][:, ts(m0 + i, P)],
                            identb,
                        )
                    trv = trm[:, 0 : 4 * (VW + 1)].rearrange(
                        "p (m w) -> p m w", w=VW + 1
                    )
                    rec = recp.tile([P, 4, 1], FP32, tag="rec")

                    nc.vector.reciprocal(rec, trv[:, :, DH:VW])
                    nc.vector.tensor_mul(
                        out_sb[:, m0 : m0 + 4, h * DH : (h + 1) * DH],
                        trv[:, :, 0:DH],
                        rec.broadcast_to([P, 4, DH]),
                    )

                return emit

            order = [(1, j) for j in range(4)] + [(2, j) for j in range(NS)]
            stream = [smm_unit(*order[0]), smm_unit(*order[1])]
            for i in range(2, len(order)):
                pp, jj = order[i - 2]
                stream += [pv_unit(pp, hA, jj), pv_unit(pp, hB, jj)]
                if (pp, jj) == (1, 3):
                    stream += [ctx_copy_unit(1, hA), ctx_copy_unit(1, hB)]
                stream.append(smm_unit(*order[i]))
            for pp, jj in order[-2:]:
                stream += [pv_unit(pp, hA, jj), pv_unit(pp, hB, jj)]
            stream += [ctx_copy_unit(2, hA), ctx_copy_unit(2, hB)]
            norms = [
                norm_unit(hA, 0),
                norm_unit(hA, 4),
                norm_unit(hB, 0),
                norm_unit(hB, 4),
            ]
            return stream, norms

        # Software pipeline: group g's projections emit interleaved with
        # group g-1's attention so ScalarE exp always overlaps PE matmuls;
        # each pair's normalization is deferred into the next pair's stream.
        pending_norms = []
        for gi in range(NG + 1):
            att = []
            if gi >= 1:
                stream, norms = attention_pair_units(gi - 1)
                # fold the previous pair's norm units into this stream
                for k, nu in enumerate(pending_norms):
                    stream.insert(4 + 5 * k, nu)
                att = stream
                pending_norms = norms
            prj = proj_units(gi) if gi < NG else []
            na, np_ = len(att), len(prj)
            if np_ == 2 and na:
                # place Q/K projections at the pair start: their matmuls fill
                # the PE bubble while the first exp chains drain
                for i, u in enumerate([prj[0]] + att[:3] + [prj[1]] + att[3:]):
                    u()
            else:
                # proportional round-robin merge
                ia = ip = 0
                while ia < na or ip < np_:
                    if ip * max(na, 1) <= ia * max(np_, 1):
                        if ip < np_:
                            prj[ip]()
                            ip += 1
                        else:
                            att[ia]()
                            ia += 1
                    else:
                        if ia < na:
                            att[ia]()
                            ia += 1
                        else:
                            prj[ip]()
                            ip += 1
        for nu in pending_norms:
            nu()

        # output DMAs: first half (heads 0-5), then heads 6-9, then the last
        # pair alone — so the flush gated on the final pair's norms is only
        # 128 columns; spread over three DGE rings so no single ring's
        # descriptor generation serializes it
        rings = (nc.sync, nc.scalar, nc.gpsimd)
        di = 0
        for c0, cn in ((0, 6 * DH), (6 * DH, 4 * DH), (10 * DH, 2 * DH)):
            for ns in range(NS):
                rings[di % 3].dma_start(
                    out=out_t[:, ns, c0 : c0 + cn],
                    in_=out_sb[:, ns, c0 : c0 + cn],
                )
                di += 1


_NC = {}


def build_nc(reps=1):
    """Build + compile the per-core Bass program once per process.

    reps > 1 emits the body multiple times with all-engine barriers between
    repetitions — used only for marginal-time measurement in test harnesses.
    """
    if reps in _NC:
        return _NC[reps]
    nc = bacc.Bacc("TRN2", target_bir_lowering=False, debug=False)
    ins = {
        "x": nc.dram_tensor("x", [S, D], FP32, kind="ExternalInput").ap(),
        "Wq": nc.dram_tensor("Wq", [H, D, DH], FP32, kind="ExternalInput").ap(),
        "Wk": nc.dram_tensor("Wk", [H, D, DH], FP32, kind="ExternalInput").ap(),
        "Wv": nc.dram_tensor("Wv", [H, D, DH], FP32, kind="ExternalInput").ap(),
    }
    outs = {"out": nc.dram_tensor("out", [S, D], FP32, kind="ExternalOutput").ap()}
    with tile.TileContext(nc) as tc:
        for i in range(reps):
            if i:
                tc.strict_bb_all_engine_barrier()
            _build_tile_kernel(tc, outs, ins)
    nc.compile()
    _NC[reps] = nc
    return nc


def make_in_maps(x, Wq, Wk, Wv):
    x = np.ascontiguousarray(x, dtype=np.float32)
    Wq = np.ascontiguousarray(Wq, dtype=np.float32)
    Wk = np.ascontiguousarray(Wk, dtype=np.float32)
    Wv = np.ascontiguousarray(Wv, dtype=np.float32)
    return [
        {"x": np.ascontiguousarray(x[b]), "Wq": Wq, "Wk": Wk, "Wv": Wv}
        for b in range(B)
    ]


def kernel(x, Wq, Wk, Wv):
    nc = build_nc()
    res = run_bass_kernel_spmd(nc, make_in_maps(x, Wq, Wk, Wv), list(range(N_CORES)))
    return np.stack([res.results[b]["out"] for b in range(B)], axis=0)
